# revision 6
# baseline (speedup 1.0000x reference)
"""Trainium2 Bass kernel for the nn_Attention problem.

Math (per flattened batch row b of x):
    qkv = x @ W_pre + b_pre                  # [B, 3*16*128]
    q,k,v -> [B, 16, 128]
    S = softmax(q k^T / sqrt(128), axis=g)   # [B, 16, 16]
    out = (sum_h S_h) . v @ W_proj + 16*b_proj
        = (sigma^T V) @ W_proj + 16*b_proj   with sigma[g] = sum_h S[h, g]

Implementation notes:
  - Data-parallel over 8 NeuronCores: 4096 rows/core (32 tiles of 128 rows).
  - The dominant qkv matmul runs in fp8(e4m3) DoubleRow perf mode (0.5
    PE cycles/row, two 128-deep k-tiles per instruction) with a 3-term
    error-compensated split: x ~ x8 + xlo and W_pre ~ Whi + Wlo (all
    e4m3, quantized host-side), qkv ~= x8(Whi + Wlo) + xlo*Whi.  End-to-end
    this matches bf16 accuracy (rel err ~4e-3 vs fp32 reference) at 6/8 of
    the bf16 PE cost.  x is scaled by 16 and W by 64 before quantization, so
    PSUM holds 1024*qkv; the scale is folded into the exp() activation scale
    and into W_proj host-side (softmax sigma weights are scale-free).
  - x8/xlo are shipped pre-transposed ([d, k-chunk, row] per tile) so the
    DoubleRow stationary operand is a contiguous [128, 2, 128] slice.
  - Attention processed in groups of 8 rows so the 128x128 PE array is full:
    stationary/moving operands are contiguous [d=128, (8 rows x 16 heads)]
    slices of PE-transposed, b-major Q/K. Cross-row score blocks are zeroed
    by a multiplicative block-diagonal mask fused with the softmax-denominator
    row sum on the DVE/Pool.  Scores for 4 groups share one PSUM bank so a
    single ACT exp() instruction covers 512 columns.
  - sigma = E^T r computed on the PE (contract over the (row,head) partition
    dim), scattered to a block-diagonal [128, 8] operand, and contracted with
    V8 (v rows expanded onto partitions via an SBUF->SBUF reshape DMA) to
    accumulate C^T directly; C^T is then the stationary operand of the final
    projection matmul.
  - Elementwise/PSUM-drain work is spread across DVE, Pool (gpsimd) and ACT
    so no single engine approaches the PE's critical path.
"""

import sys

import numpy as np

for _p in ("/opt/trn_rl_repo",):
    if _p not in sys.path:
        sys.path.insert(0, _p)

import ml_dtypes  # noqa: E402

BF16 = ml_dtypes.bfloat16
E4M3 = ml_dtypes.float8_e4m3

HEAD_NUM = 16
HEAD_DIM = 128
INPUT_DIM = 1024
OUTPUT_DIM = 1024
QKV_DIM = 3 * HEAD_NUM * HEAD_DIM  # 6144
N_CORES = 8
B_TOTAL = 64 * 512
ROWS_PER_CORE = B_TOTAL // N_CORES  # 4096
P = 128
XSCALE = 16.0
WSCALE = 64.0
QKV_SCALE = XSCALE * WSCALE  # 1024

_PROG = None


def _build_program(n_tiles=ROWS_PER_CORE // P, repeats=1):
    from contextlib import ExitStack

    import concourse.tile as tile
    from concourse import bacc, mybir

    dt = mybir.dt
    Alu = mybir.AluOpType
    Act = mybir.ActivationFunctionType
    DR = mybir.MatmulPerfMode.DoubleRow

    rows = n_tiles * P
    nc = bacc.Bacc("TRN2", target_bir_lowering=False, debug=False,
                   num_devices=N_CORES)

    # x8/xlo are pre-transposed host-side: row t*128+d, col m*128+b holds
    # x[t*128+b, m*128+d] (quantized e4m3, scaled by 16).
    x8_d = nc.dram_tensor("x8t", [rows, INPUT_DIM], dt.float8e4,
                          kind="ExternalInput")
    xlo_d = nc.dram_tensor("xlot", [rows, INPUT_DIM], dt.float8e4,
                           kind="ExternalInput")
    # W halves: row p, col k*6144+j holds W*64 quantized, k = p's chunk.
    whi_d = nc.dram_tensor("w_hi", [P, (INPUT_DIM // P) * QKV_DIM],
                           dt.float8e4, kind="ExternalInput")
    wlo_d = nc.dram_tensor("w_lo", [P, (INPUT_DIM // P) * QKV_DIM],
                           dt.float8e4, kind="ExternalInput")
    bpre_d = nc.dram_tensor("b_pre_rep", [P, QKV_DIM], dt.bfloat16,
                            kind="ExternalInput")
    wproj_d = nc.dram_tensor("w_proj", [HEAD_DIM, OUTPUT_DIM], dt.bfloat16,
                             kind="ExternalInput")
    bproj_d = nc.dram_tensor("b_proj16_rep", [P, OUTPUT_DIM], dt.float32,
                             kind="ExternalInput")
    mask01_d = nc.dram_tensor("mask01", [P, P], dt.bfloat16,
                              kind="ExternalInput")
    mask8_d = nc.dram_tensor("mask8", [P, 8], dt.bfloat16,
                             kind="ExternalInput")
    ident_d = nc.dram_tensor("ident", [P, P], dt.bfloat16,
                             kind="ExternalInput")
    out_d = nc.dram_tensor("out", [rows, OUTPUT_DIM], dt.float32,
                           kind="ExternalOutput")

    KC = INPUT_DIM // P          # 8 contraction chunks (4 DoubleRow pairs)
    KP = KC // 2                 # 4 k-pairs
    NCHUNK = 512                 # psum free width
    SWEEP = 3                    # psum banks used by the qkv matmul
    NSWEEPS = QKV_DIM // (SWEEP * NCHUNK)  # 4
    GROUPS = P // 8              # 16 groups of 8 rows per tile
    GB = 4                       # score groups per psum bank / exp batch
    INV_SQRT_D = 1.0 / float(np.sqrt(HEAD_DIM))
    EXP_SCALE = INV_SQRT_D / (QKV_SCALE * QKV_SCALE)

    with tile.TileContext(nc) as tc, ExitStack() as ctx:
        consts = ctx.enter_context(tc.tile_pool(name="consts", bufs=1))
        xt_pool = ctx.enter_context(tc.tile_pool(name="xt", bufs=2))
        qkv_pool = ctx.enter_context(tc.tile_pool(name="qkv", bufs=2))
        qt_pool = ctx.enter_context(tc.tile_pool(name="qt", bufs=2))
        kt_pool = ctx.enter_context(tc.tile_pool(name="kt", bufs=2))
        v8_pool = ctx.enter_context(tc.tile_pool(name="v8", bufs=2))
        att_pool = ctx.enter_context(tc.tile_pool(name="att", bufs=4))
        ct_pool = ctx.enter_context(tc.tile_pool(name="ct", bufs=2))
        out_pool = ctx.enter_context(tc.tile_pool(name="outp", bufs=2))

        qkv_ps = ctx.enter_context(
            tc.tile_pool(name="qkv_ps", bufs=SWEEP, space="PSUM"))
        z_ps = ctx.enter_context(tc.tile_pool(name="z_ps", bufs=2, space="PSUM"))
        tp_ps = ctx.enter_context(tc.tile_pool(name="tp_ps", bufs=2, space="PSUM"))
        ct_ps = ctx.enter_context(tc.tile_pool(name="ct_ps", bufs=1, space="PSUM"))

        # tile 0's x loads go first so the first qkv matmul only waits on
        # them plus the first W chunk, not the whole 12.6MB weight load.
        xt0 = xt_pool.tile([P, KC, P], dt.float8e4, name="xt")
        xl0 = xt_pool.tile([P, KC, P], dt.float8e4, name="xl")
        nc.sync.dma_start(xt0[:], x8_d[0:P, :].rearrange(
            "d (m b) -> d m b", b=P))
        nc.sync.dma_start(xl0[:], xlo_d[0:P, :].rearrange(
            "d (m b) -> d m b", b=P))
        preloaded_xt = {0: (xt0, xl0)}

        # ---- resident constants ----
        whi_sb = consts.tile([P, KC, QKV_DIM], dt.float8e4)
        wlo_sb = consts.tile([P, KC, QKV_DIM], dt.float8e4)
        for k in range(KC):
            eng = nc.sync if k % 2 == 0 else nc.scalar
            eng.dma_start(whi_sb[:, k, :],
                          whi_d[:, k * QKV_DIM:(k + 1) * QKV_DIM])
            eng = nc.scalar if k % 2 == 0 else nc.sync
            eng.dma_start(wlo_sb[:, k, :],
                          wlo_d[:, k * QKV_DIM:(k + 1) * QKV_DIM])
        wproj_sb = consts.tile([P, OUTPUT_DIM], dt.bfloat16)
        nc.sync.dma_start(wproj_sb[:], wproj_d[:, :])
        bpre_sb = consts.tile([P, QKV_DIM], dt.bfloat16)
        nc.sync.dma_start(bpre_sb[:], bpre_d[:, :])
        bproj_sb = consts.tile([P, OUTPUT_DIM], dt.float32)
        nc.sync.dma_start(bproj_sb[:], bproj_d[:, :])
        mask01_sb = consts.tile([P, P], dt.bfloat16)
        nc.sync.dma_start(mask01_sb[:], mask01_d[:, :])
        mask8_sb = consts.tile([P, 8], dt.bfloat16)
        nc.sync.dma_start(mask8_sb[:], mask8_d[:, :])
        ident_sb = consts.tile([P, P], dt.bfloat16)
        nc.sync.dma_start(ident_sb[:], ident_d[:, :])

        state = {}

        def front_gen(t):
            """Emits tile t's qkv matmuls, yielding after each one so the
            caller can interleave the previous tile's attention steps into
            the PE queue; finishes with transposes + the V8 reshape."""
            r0 = t * P
            if t in preloaded_xt:
                xt, xl = preloaded_xt.pop(t)
            else:
                xt = xt_pool.tile([P, KC, P], dt.float8e4, name="xt")
                xl = xt_pool.tile([P, KC, P], dt.float8e4, name="xl")
                nc.sync.dma_start(xt[:], x8_d[r0:r0 + P, :].rearrange(
                    "d (m b) -> d m b", b=P))
                nc.sync.dma_start(xl[:], xlo_d[r0:r0 + P, :].rearrange(
                    "d (m b) -> d m b", b=P))

            qkv_sb = qkv_pool.tile([P, QKV_DIM], dt.bfloat16, name="qkv_sb")
            dr_terms = ((xt, whi_sb), (xl, whi_sb), (xt, wlo_sb))
            for s in range(NSWEEPS):
                chunks = []
                for c in range(SWEEP):
                    chunks.append(qkv_ps.tile([P, NCHUNK], dt.float32,
                                              name="qkvps", tag="qkvps"))
                for m in range(KP):
                    for ti, (xop, wop) in enumerate(dr_terms):
                        for c in range(SWEEP):
                            j0 = (s * SWEEP + c) * NCHUNK
                            nc.tensor.matmul(
                                chunks[c][:],
                                lhsT=xop[:, 2 * m:2 * m + 2, :],
                                rhs=wop[:, 2 * m:2 * m + 2, j0:j0 + NCHUNK],
                                start=(m == 0 and ti == 0),
                                stop=(m == KP - 1 and ti == len(dr_terms) - 1),
                                perf_mode=DR,
                            )
                            yield
                for c in range(SWEEP):
                    j0 = (s * SWEEP + c) * NCHUNK
                    # psum fp32 + b_pre -> bf16 SBUF (GPSIMD cannot read
                    # PSUM, so these all live on the DVE)
                    eng = nc.vector
                    eng.tensor_tensor(
                        qkv_sb[:, j0:j0 + NCHUNK],
                        chunks[c][:],
                        bpre_sb[:, j0:j0 + NCHUNK],
                        Alu.add,
                    )

            # transposed q/k in b-major layout: qt[d, b*16+h] = q[b, h*128+d],
            # so every 8-row group is a contiguous 128-column slice (matmul
            # operands must have a single free dimension). 8 head-transposes
            # share one PSUM bank, drained by a single strided copy.
            qt = qt_pool.tile([P, P, HEAD_NUM], dt.bfloat16, name="qt")
            kt = kt_pool.tile([P, P, HEAD_NUM], dt.bfloat16, name="kt")
            for qk in range(2):
                src_off = 2048 * qk
                dst = (qt, kt)[qk]
                for hb in range(2):
                    h0 = 8 * hb
                    tpb = tp_ps.tile([P, 8 * P], dt.bfloat16, name="tpb",
                                     tag="tpb")
                    for hl in range(8):
                        nc.tensor.transpose(
                            tpb[:, hl * P:(hl + 1) * P],
                            qkv_sb[:, src_off + (h0 + hl) * P:
                                   src_off + (h0 + hl + 1) * P],
                            ident_sb[:])
                    (nc.scalar.copy if (qk + hb) % 2 == 0
                     else nc.vector.tensor_copy)(
                        dst[:, :, h0:h0 + 8],
                        tpb.rearrange("d (h b) -> d b h", b=P))

            # v8[(b_loc, g), grp, d] = v[8*grp + b_loc, g*128 + d]
            v8 = v8_pool.tile([P, GROUPS, HEAD_DIM], dt.bfloat16, name="v8")
            for g in range(GROUPS):
                nc.sync.dma_start(
                    v8[:, g, :],
                    qkv_sb[8 * g:8 * g + 8, 4096:6144].rearrange(
                        "b (g d) -> b g d", d=HEAD_DIM),
                )
            state[t] = (qt, kt, v8)

        def attention_steps(t):
            """Returns (steps, tail): `steps` are callables interleaved with
            the next tile's qkv matmuls.  Scores are produced in batches of
            GB=4 groups sharing one PSUM bank so exp() covers 512 columns per
            ACT instruction; mm2/mm3 trail the batch."""
            qt, kt, v8 = state.pop(t)
            r0 = t * P
            ct_box = {}
            zbank, ems, dens, rbfs, sds = {}, {}, {}, {}, {}

            def mm1(g):
                if g == 0:
                    ct_box["ct"] = ct_ps.tile([P, P], dt.float32, name="ct")
                b = g // GB
                gi = g % GB
                if gi == 0:
                    zbank[b] = z_ps.tile([P, GB * P], dt.float32, name="z4",
                                         tag="z4")
                b0 = 8 * g
                # scores for 8 rows x all head pairs: [(b,h), (b',g)]
                nc.tensor.matmul(
                    zbank[b][:, gi * P:(gi + 1) * P],
                    lhsT=qt[:, b0:b0 + 8, :].rearrange("d b h -> d (b h)"),
                    rhs=kt[:, b0:b0 + 8, :].rearrange("d b h -> d (b h)"),
                    start=True,
                    stop=True,
                )

            def softmax_batch(b):
                # one exp() over the whole 4-group bank, then per-group
                # masked row-sums (alternating DVE/Pool), one batched
                # reciprocal and one fp32->bf16 copy.
                em4 = att_pool.tile([P, GB * P], dt.bfloat16, tag="em4",
                                    name="em4")
                nc.scalar.activation(em4[:], zbank[b][:], Act.Exp,
                                     scale=EXP_SCALE)
                den4 = att_pool.tile([P, GB], dt.float32, tag="den4",
                                     name="den4")
                emm4 = att_pool.tile([P, GB * P], dt.bfloat16, tag="emm4",
                                     name="emm4")
                for gi in range(GB):
                    eng = nc.vector
                    eng.scalar_tensor_tensor(
                        out=emm4[:, gi * P:(gi + 1) * P],
                        in0=em4[:, gi * P:(gi + 1) * P],
                        scalar=1.0,
                        in1=mask01_sb[:],
                        op0=Alu.mult, op1=Alu.mult,
                        accum_out=den4[:, gi:gi + 1])
                    ems[b * GB + gi] = emm4
                r4 = att_pool.tile([P, GB], dt.float32, tag="r4", name="r4")
                nc.vector.reciprocal(r4[:], den4[:])
                rbf4 = att_pool.tile([P, GB], dt.bfloat16, tag="rbf4",
                                     name="rbf4")
                nc.vector.tensor_copy(rbf4[:], r4[:])
                for gi in range(GB):
                    rbfs[b * GB + gi] = rbf4[:, gi:gi + 1]
                dens[b] = (den4, em4)

            def mm2(g):
                b = g // GB
                gi = g % GB
                em = ems.pop(g)
                # sigma[(b,g)] = sum_{(b,h)} em[(b,h),(b,g)] * r[(b,h)]
                sig = zbank[b][:, gi * P:gi * P + 1]  # dead z psum slot
                nc.tensor.matmul(sig, lhsT=em[:, gi * P:(gi + 1) * P],
                                 rhs=rbfs.pop(g), start=True, stop=True)
                sd = att_pool.tile([P, 8], dt.bfloat16, tag="sd", name="sd")
                nc.vector.tensor_scalar(sd[:], mask8_sb[:], sig, None,
                                        Alu.mult)
                sds[g] = sd

            def mm3(g):
                b0 = 8 * g
                # C^T[:, rows of this group] += sigma-weighted V rows
                nc.tensor.matmul(ct_box["ct"][:, b0:b0 + 8],
                                 lhsT=v8[:, g, :],
                                 rhs=sds.pop(g)[:], start=True, stop=True)
                if g % GB == GB - 1:
                    zbank.pop(g // GB, None)

            steps = []
            for g in range(GROUPS + 5):
                def step(g=g):
                    # mm2 lags mm1 by 4 (its exp batch), mm3 by one more.
                    if 4 <= g < GROUPS + 4:
                        mm2(g - 4)
                    if g < GROUPS:
                        mm1(g)
                        if g % GB == GB - 1:
                            softmax_batch(g // GB)
                    if g >= 5:
                        mm3(g - 5)
                steps.append(step)

            def tail():
                ct_sb = ct_pool.tile([P, P], dt.bfloat16, name="ct_sb")
                nc.scalar.copy(ct_sb[:], ct_box["ct"][:])
                out_sb = out_pool.tile([P, OUTPUT_DIM], dt.float32,
                                       name="out_sb")
                for c in range(OUTPUT_DIM // NCHUNK):
                    o_ps = qkv_ps.tile([P, NCHUNK], dt.float32, name="o_ps",
                                       tag="qkvps")
                    nc.tensor.matmul(
                        o_ps[:],
                        lhsT=ct_sb[:],
                        rhs=wproj_sb[:, c * NCHUNK:(c + 1) * NCHUNK],
                        start=True,
                        stop=True,
                    )
                    nc.vector.tensor_tensor(
                        out_sb[:, c * NCHUNK:(c + 1) * NCHUNK],
                        o_ps[:],
                        bproj_sb[:, c * NCHUNK:(c + 1) * NCHUNK],
                        Alu.add,
                    )
                nc.sync.dma_start(out_d[r0:r0 + P, :], out_sb[:])

            return steps, tail

        # software pipeline: tile t's attention steps are interleaved into
        # tile t+1's qkv matmul stream so the PE stays busy while the
        # softmax chains drain on ACT/DVE/Pool. repeats>1 re-runs the whole
        # pass (same outputs) for benchmarking.
        prev = None
        for _r in range(repeats):
            for t in range(n_tiles):
                steps, tail = attention_steps(prev) if prev is not None \
                    else ([], None)
                si = 0
                yi = 0
                for _ in front_gen(t):
                    yi += 1
                    if si < len(steps) and yi % 6 == 0:
                        steps[si]()
                        si += 1
                while si < len(steps):
                    steps[si]()
                    si += 1
                if tail is not None:
                    tail()
                prev = t
        steps, tail = attention_steps(prev)
        for s in steps:
            s()
        tail()

    nc.compile()
    return nc


def _host_inputs(x, W_pre, b_pre, W_proj, b_proj, n_tiles=ROWS_PER_CORE // P,
                 n_cores=N_CORES):
    rows = n_tiles * P
    xf = np.ascontiguousarray(np.asarray(x, dtype=np.float32)
                              .reshape(-1, INPUT_DIM))
    x16 = xf * np.float32(XSCALE)
    x8 = x16.astype(E4M3)
    xlo = (x16 - x8.astype(np.float32)).astype(E4M3)

    def xt_layout(a, c):
        # [rows, 1024] -> [t, d, m, b] -> [rows, 1024] (see x8_d comment)
        blk = a[c * rows:(c + 1) * rows].reshape(n_tiles, P, INPUT_DIM // P, P)
        return np.ascontiguousarray(
            blk.transpose(0, 3, 2, 1).reshape(rows, INPUT_DIM))

    w64 = np.asarray(W_pre, dtype=np.float32) * np.float32(WSCALE)
    whi = w64.astype(E4M3)
    wlo = (w64 - whi.astype(np.float32)).astype(E4M3)

    def w_layout(a):
        return np.ascontiguousarray(
            a.reshape(INPUT_DIM // P, P, QKV_DIM).transpose(1, 0, 2)
            .reshape(P, (INPUT_DIM // P) * QKV_DIM))

    whi_l = w_layout(whi)
    wlo_l = w_layout(wlo)
    wproj16 = (np.asarray(W_proj, dtype=np.float32)
               / np.float32(QKV_SCALE)).astype(BF16)
    bpre_rep = np.broadcast_to(
        (np.float32(QKV_SCALE) * np.asarray(b_pre, dtype=np.float32))
        .astype(BF16)[None, :], (P, QKV_DIM)).copy()
    bproj_rep = np.broadcast_to(
        (16.0 * np.asarray(b_proj, dtype=np.float32))[None, :],
        (P, OUTPUT_DIM)).copy()
    pi = np.arange(P)[:, None] // HEAD_NUM
    fi = np.arange(P)[None, :] // HEAD_NUM
    mask01 = (pi == fi).astype(BF16)
    mask8 = (np.arange(P)[:, None] // HEAD_NUM
             == np.arange(8)[None, :]).astype(BF16)
    ident = np.eye(P).astype(BF16)

    in_maps = []
    for c in range(n_cores):
        in_maps.append({
            "x8t": xt_layout(x8, c),
            "xlot": xt_layout(xlo, c),
            "w_hi": whi_l,
            "w_lo": wlo_l,
            "b_pre_rep": bpre_rep,
            "w_proj": wproj16,
            "b_proj16_rep": bproj_rep,
            "mask01": mask01,
            "mask8": mask8,
            "ident": ident,
        })
    return in_maps


def kernel(x, W_pre, b_pre, W_proj, b_proj):
    global _PROG
    from concourse.bass_utils import run_bass_kernel_spmd

    if _PROG is None:
        _PROG = _build_program()

    in_maps = _host_inputs(x, W_pre, b_pre, W_proj, b_proj)
    res = run_bass_kernel_spmd(_PROG, in_maps, list(range(N_CORES)))
    out = np.concatenate([res.results[c]["out"] for c in range(N_CORES)],
                         axis=0)
    return out.reshape(*np.asarray(x).shape[:-1], OUTPUT_DIM).astype(np.float32)


# revision 7
# speedup vs baseline: 3.1585x; 3.1585x over previous
"""Trainium2 Bass kernel for the nn_Attention problem.

Math (per flattened batch row b of x):
    qkv = x @ W_pre + b_pre                  # [B, 3*16*128]
    q,k,v -> [B, 16, 128]
    S = softmax(q k^T / sqrt(128), axis=g)   # [B, 16, 16]
    out = (sum_h S_h) . v @ W_proj + 16*b_proj
        = (sigma^T V) @ W_proj + 16*b_proj   with sigma[g] = sum_h S[h, g]

Implementation notes:
  - Data-parallel over 8 NeuronCores: 4096 rows/core (32 tiles of 128 rows).
  - bf16 matmuls with fp32 PSUM accumulation; softmax in fp32.  (fp8
    variants were measured on this hardware: plain fp8 runs ~1.5x faster
    per matmul and DoubleRow fuses 2 k-tiles per instruction at bf16
    instruction cost, but single-fp8 operands cost ~2.6% rms input error
    which lands at ~0.026 rel err end-to-end (budget 2e-2), and any
    error-compensated multi-term split needs >= 3 products per k-chunk,
    which is slower than the 1-product bf16 path.  bf16 is the optimum
    here.)
  - Attention processed in groups of 8 rows so the 128x128 PE array is full:
    stationary/moving operands are contiguous [d=128, (8 rows x 16 heads)]
    slices of PE-transposed, b-major Q/K. Cross-row score blocks are zeroed
    by a multiplicative block-diagonal mask fused with the softmax-denominator
    row sum on the DVE.  Scores for 4 groups share one PSUM bank so a single
    ACT exp() instruction covers 512 columns; reciprocals and bf16 casts are
    batched 4-wide as well, quartering the small-instruction overhead of the
    softmax chain vs one-group-at-a-time processing.
  - sigma = E^T r computed on the PE (contract over the (row,head) partition
    dim), scattered to a block-diagonal [128, 8] operand, and contracted with
    V8 (v rows expanded onto partitions via an SBUF->SBUF reshape DMA) to
    accumulate C^T directly; C^T is then the stationary operand of the final
    projection matmul.
"""

import sys

import numpy as np

for _p in ("/opt/trn_rl_repo",):
    if _p not in sys.path:
        sys.path.insert(0, _p)

import ml_dtypes  # noqa: E402

BF16 = ml_dtypes.bfloat16

HEAD_NUM = 16
HEAD_DIM = 128
INPUT_DIM = 1024
OUTPUT_DIM = 1024
QKV_DIM = 3 * HEAD_NUM * HEAD_DIM  # 6144
N_CORES = 8
B_TOTAL = 64 * 512
ROWS_PER_CORE = B_TOTAL // N_CORES  # 4096
P = 128

_PROG = None


def _build_program(n_tiles=ROWS_PER_CORE // P, repeats=1):
    from contextlib import ExitStack

    import concourse.tile as tile
    from concourse import bacc, mybir

    dt = mybir.dt
    Alu = mybir.AluOpType
    Act = mybir.ActivationFunctionType

    rows = n_tiles * P
    nc = bacc.Bacc("TRN2", target_bir_lowering=False, debug=False,
                   num_devices=N_CORES)

    x_d = nc.dram_tensor("x", [rows, INPUT_DIM], dt.bfloat16,
                         kind="ExternalInput")
    wpre_d = nc.dram_tensor("w_pre", [INPUT_DIM, QKV_DIM], dt.bfloat16,
                            kind="ExternalInput")
    bpre_d = nc.dram_tensor("b_pre_rep", [P, QKV_DIM], dt.bfloat16,
                            kind="ExternalInput")
    wproj_d = nc.dram_tensor("w_proj", [HEAD_DIM, OUTPUT_DIM], dt.bfloat16,
                             kind="ExternalInput")
    bproj_d = nc.dram_tensor("b_proj16_rep", [P, OUTPUT_DIM], dt.float32,
                             kind="ExternalInput")
    mask01_d = nc.dram_tensor("mask01", [P, P], dt.bfloat16,
                              kind="ExternalInput")
    mask8_d = nc.dram_tensor("mask8", [P, 8], dt.bfloat16,
                             kind="ExternalInput")
    ident_d = nc.dram_tensor("ident", [P, P], dt.bfloat16,
                             kind="ExternalInput")
    out_d = nc.dram_tensor("out", [rows, OUTPUT_DIM], dt.float32,
                           kind="ExternalOutput")

    KC = INPUT_DIM // P          # 8 contraction chunks
    NCHUNK = 512                 # psum free width
    SWEEP = 3                    # psum banks used by the qkv matmul
    NSWEEPS = QKV_DIM // (SWEEP * NCHUNK)  # 4
    GROUPS = P // 8              # 16 groups of 8 rows per tile
    GB = 4                       # score groups per psum bank / exp batch
    INV_SQRT_D = 1.0 / float(np.sqrt(HEAD_DIM))

    with tile.TileContext(nc) as tc, ExitStack() as ctx:
        consts = ctx.enter_context(tc.tile_pool(name="consts", bufs=1))
        xt_pool = ctx.enter_context(tc.tile_pool(name="xt", bufs=2))
        qkv_pool = ctx.enter_context(tc.tile_pool(name="qkv", bufs=2))
        qt_pool = ctx.enter_context(tc.tile_pool(name="qt", bufs=2))
        kt_pool = ctx.enter_context(tc.tile_pool(name="kt", bufs=2))
        v8_pool = ctx.enter_context(tc.tile_pool(name="v8", bufs=2))
        att_pool = ctx.enter_context(tc.tile_pool(name="att", bufs=4))
        ct_pool = ctx.enter_context(tc.tile_pool(name="ct", bufs=2))
        out_pool = ctx.enter_context(tc.tile_pool(name="outp", bufs=2))

        qkv_ps = ctx.enter_context(
            tc.tile_pool(name="qkv_ps", bufs=SWEEP, space="PSUM"))
        z_ps = ctx.enter_context(tc.tile_pool(name="z_ps", bufs=2, space="PSUM"))
        tp_ps = ctx.enter_context(tc.tile_pool(name="tp_ps", bufs=2, space="PSUM"))
        ct_ps = ctx.enter_context(tc.tile_pool(name="ct_ps", bufs=1, space="PSUM"))

        # tile 0's x^T load goes first so the first qkv matmul only waits
        # on it plus the first W chunk, not the whole 12.6MB weight load.
        xt0 = xt_pool.tile([P, KC, P], dt.bfloat16, name="xt")
        nc.sync.dma_start_transpose(xt0[:], x_d[0:P, :])
        preloaded_xt = {0: xt0}

        # ---- resident constants ----
        wpre_sb = consts.tile([P, KC, QKV_DIM], dt.bfloat16)
        for k in range(KC):
            eng = nc.sync if k % 2 == 0 else nc.scalar
            eng.dma_start(wpre_sb[:, k, :], wpre_d[k * P:(k + 1) * P, :])
        wproj_sb = consts.tile([P, OUTPUT_DIM], dt.bfloat16)
        nc.sync.dma_start(wproj_sb[:], wproj_d[:, :])
        bpre_sb = consts.tile([P, QKV_DIM], dt.bfloat16)
        nc.sync.dma_start(bpre_sb[:], bpre_d[:, :])
        bproj_sb = consts.tile([P, OUTPUT_DIM], dt.float32)
        nc.sync.dma_start(bproj_sb[:], bproj_d[:, :])
        mask01_sb = consts.tile([P, P], dt.bfloat16)
        nc.sync.dma_start(mask01_sb[:], mask01_d[:, :])
        mask8_sb = consts.tile([P, 8], dt.bfloat16)
        nc.sync.dma_start(mask8_sb[:], mask8_d[:, :])
        ident_sb = consts.tile([P, P], dt.bfloat16)
        nc.sync.dma_start(ident_sb[:], ident_d[:, :])

        state = {}

        def front_gen(t):
            """Emits tile t's qkv matmuls, yielding after each one so the
            caller can interleave the previous tile's attention steps into
            the PE queue; finishes with transposes + the V8 reshape."""
            r0 = t * P
            # x^T tiles via XBAR DMA transpose: xt[d, kc, b] = x[r0+b, kc*128+d]
            if t in preloaded_xt:
                xt = preloaded_xt.pop(t)
            else:
                xt = xt_pool.tile([P, KC, P], dt.bfloat16, name="xt")
                nc.sync.dma_start_transpose(xt[:], x_d[r0:r0 + P, :])

            qkv_sb = qkv_pool.tile([P, QKV_DIM], dt.bfloat16, name="qkv_sb")
            for s in range(NSWEEPS):
                chunks = []
                for c in range(SWEEP):
                    chunks.append(qkv_ps.tile([P, NCHUNK], dt.float32,
                                              name="qkvps", tag="qkvps"))
                for k in range(KC):
                    for c in range(SWEEP):
                        j0 = (s * SWEEP + c) * NCHUNK
                        nc.tensor.matmul(
                            chunks[c][:],
                            lhsT=xt[:, k, :],
                            rhs=wpre_sb[:, k, j0:j0 + NCHUNK],
                            start=(k == 0),
                            stop=(k == KC - 1),
                        )
                        yield
                for c in range(SWEEP):
                    j0 = (s * SWEEP + c) * NCHUNK
                    # psum fp32 + b_pre -> bf16 SBUF
                    nc.vector.tensor_tensor(
                        qkv_sb[:, j0:j0 + NCHUNK],
                        chunks[c][:],
                        bpre_sb[:, j0:j0 + NCHUNK],
                        Alu.add,
                    )

            # transposed q/k in b-major layout: qt[d, b*16+h] = q[b, h*128+d],
            # so every 8-row group is a contiguous 128-column slice (matmul
            # operands must have a single free dimension). 8 head-transposes
            # share one PSUM bank, drained by a single strided copy.
            qt = qt_pool.tile([P, P, HEAD_NUM], dt.bfloat16, name="qt")
            kt = kt_pool.tile([P, P, HEAD_NUM], dt.bfloat16, name="kt")
            for qk in range(2):
                src_off = 2048 * qk
                dst = (qt, kt)[qk]
                for hb in range(2):
                    h0 = 8 * hb
                    tpb = tp_ps.tile([P, 8 * P], dt.bfloat16, name="tpb",
                                     tag="tpb")
                    for hl in range(8):
                        nc.tensor.transpose(
                            tpb[:, hl * P:(hl + 1) * P],
                            qkv_sb[:, src_off + (h0 + hl) * P:
                                   src_off + (h0 + hl + 1) * P],
                            ident_sb[:])
                    (nc.scalar.copy if (qk + hb) % 2 == 0
                     else nc.vector.tensor_copy)(
                        dst[:, :, h0:h0 + 8],
                        tpb.rearrange("d (h b) -> d b h", b=P))

            # v8[(b_loc, g), grp, d] = v[8*grp + b_loc, g*128 + d]
            v8 = v8_pool.tile([P, GROUPS, HEAD_DIM], dt.bfloat16, name="v8")
            for g in range(GROUPS):
                nc.sync.dma_start(
                    v8[:, g, :],
                    qkv_sb[8 * g:8 * g + 8, 4096:6144].rearrange(
                        "b (g d) -> b g d", d=HEAD_DIM),
                )
            state[t] = (qt, kt, v8)

        def attention_steps(t):
            """Returns (steps, tail): `steps` are callables interleaved with
            the next tile's qkv matmuls.  Scores are produced in batches of
            GB=4 groups sharing one PSUM bank so exp() covers 512 columns per
            ACT instruction; mm2/mm3 trail the batch."""
            qt, kt, v8 = state.pop(t)
            r0 = t * P
            ct_box = {}
            zbank, ems, rbfs, sds = {}, {}, {}, {}

            def mm1(g):
                if g == 0:
                    ct_box["ct"] = ct_ps.tile([P, P], dt.float32, name="ct")
                b = g // GB
                gi = g % GB
                if gi == 0:
                    zbank[b] = z_ps.tile([P, GB * P], dt.float32, name="z4",
                                         tag="z4")
                b0 = 8 * g
                # scores for 8 rows x all head pairs: [(b,h), (b',g)]
                nc.tensor.matmul(
                    zbank[b][:, gi * P:(gi + 1) * P],
                    lhsT=qt[:, b0:b0 + 8, :].rearrange("d b h -> d (b h)"),
                    rhs=kt[:, b0:b0 + 8, :].rearrange("d b h -> d (b h)"),
                    start=True,
                    stop=True,
                )

            def softmax_batch(b):
                # one exp() over the whole 4-group bank, then per-group
                # masked row-sums, one batched reciprocal and one batched
                # fp32->bf16 cast.
                em4 = att_pool.tile([P, GB * P], dt.bfloat16, tag="em4",
                                    name="em4")
                nc.scalar.activation(em4[:], zbank[b][:], Act.Exp,
                                     scale=INV_SQRT_D)
                den4 = att_pool.tile([P, GB], dt.float32, tag="den4",
                                     name="den4")
                emm4 = att_pool.tile([P, GB * P], dt.bfloat16, tag="emm4",
                                     name="emm4")
                for gi in range(GB):
                    nc.vector.scalar_tensor_tensor(
                        out=emm4[:, gi * P:(gi + 1) * P],
                        in0=em4[:, gi * P:(gi + 1) * P],
                        scalar=1.0,
                        in1=mask01_sb[:],
                        op0=Alu.mult, op1=Alu.mult,
                        accum_out=den4[:, gi:gi + 1])
                    ems[b * GB + gi] = emm4
                r4 = att_pool.tile([P, GB], dt.float32, tag="r4", name="r4")
                nc.vector.reciprocal(r4[:], den4[:])
                rbf4 = att_pool.tile([P, GB], dt.bfloat16, tag="rbf4",
                                     name="rbf4")
                nc.vector.tensor_copy(rbf4[:], r4[:])
                for gi in range(GB):
                    rbfs[b * GB + gi] = rbf4[:, gi:gi + 1]

            def mm2(g):
                b = g // GB
                gi = g % GB
                em = ems.pop(g)
                # sigma[(b,g)] = sum_{(b,h)} em[(b,h),(b,g)] * r[(b,h)]
                sig = zbank[b][:, gi * P:gi * P + 1]  # dead z psum slot
                nc.tensor.matmul(sig, lhsT=em[:, gi * P:(gi + 1) * P],
                                 rhs=rbfs.pop(g), start=True, stop=True)
                sd = att_pool.tile([P, 8], dt.bfloat16, tag="sd", name="sd")
                nc.vector.tensor_scalar(sd[:], mask8_sb[:], sig, None,
                                        Alu.mult)
                sds[g] = sd

            def mm3(g):
                b0 = 8 * g
                # C^T[:, rows of this group] += sigma-weighted V rows
                nc.tensor.matmul(ct_box["ct"][:, b0:b0 + 8],
                                 lhsT=v8[:, g, :],
                                 rhs=sds.pop(g)[:], start=True, stop=True)
                if g % GB == GB - 1:
                    zbank.pop(g // GB, None)

            steps = []
            for g in range(GROUPS + 5):
                def step(g=g):
                    # mm2 lags mm1 by 4 (its exp batch), mm3 by one more.
                    if 4 <= g < GROUPS + 4:
                        mm2(g - 4)
                    if g < GROUPS:
                        mm1(g)
                        if g % GB == GB - 1:
                            softmax_batch(g // GB)
                    if g >= 5:
                        mm3(g - 5)
                steps.append(step)

            def tail():
                ct_sb = ct_pool.tile([P, P], dt.bfloat16, name="ct_sb")
                nc.scalar.copy(ct_sb[:], ct_box["ct"][:])
                out_sb = out_pool.tile([P, OUTPUT_DIM], dt.float32,
                                       name="out_sb")
                for c in range(OUTPUT_DIM // NCHUNK):
                    o_ps = qkv_ps.tile([P, NCHUNK], dt.float32, name="o_ps",
                                       tag="qkvps")
                    nc.tensor.matmul(
                        o_ps[:],
                        lhsT=ct_sb[:],
                        rhs=wproj_sb[:, c * NCHUNK:(c + 1) * NCHUNK],
                        start=True,
                        stop=True,
                    )
                    nc.vector.tensor_tensor(
                        out_sb[:, c * NCHUNK:(c + 1) * NCHUNK],
                        o_ps[:],
                        bproj_sb[:, c * NCHUNK:(c + 1) * NCHUNK],
                        Alu.add,
                    )
                nc.sync.dma_start(out_d[r0:r0 + P, :], out_sb[:])

            return steps, tail

        # software pipeline: tile t's attention steps are interleaved into
        # tile t+1's qkv matmul stream so the PE stays busy while the
        # softmax chains drain on ACT/DVE. repeats>1 re-runs the whole pass
        # (same outputs) for benchmarking.
        prev = None
        for _r in range(repeats):
            for t in range(n_tiles):
                steps, tail = attention_steps(prev) if prev is not None \
                    else ([], None)
                si = 0
                yi = 0
                for _ in front_gen(t):
                    yi += 1
                    if si < len(steps) and yi % 4 == 0:
                        steps[si]()
                        si += 1
                while si < len(steps):
                    steps[si]()
                    si += 1
                if tail is not None:
                    tail()
                prev = t
        steps, tail = attention_steps(prev)
        for s in steps:
            s()
        tail()

    nc.compile()
    return nc


def _host_inputs(x, W_pre, b_pre, W_proj, b_proj, n_tiles=ROWS_PER_CORE // P,
                 n_cores=N_CORES):
    rows = n_tiles * P
    xf = np.ascontiguousarray(np.asarray(x, dtype=np.float32)
                              .reshape(-1, INPUT_DIM)).astype(BF16)
    wpre16 = np.asarray(W_pre, dtype=np.float32).astype(BF16)
    wproj16 = np.asarray(W_proj, dtype=np.float32).astype(BF16)
    bpre_rep = np.broadcast_to(
        np.asarray(b_pre, dtype=np.float32).astype(BF16)[None, :],
        (P, QKV_DIM)).copy()
    bproj_rep = np.broadcast_to(
        (16.0 * np.asarray(b_proj, dtype=np.float32))[None, :],
        (P, OUTPUT_DIM)).copy()
    pi = np.arange(P)[:, None] // HEAD_NUM
    fi = np.arange(P)[None, :] // HEAD_NUM
    mask01 = (pi == fi).astype(BF16)
    mask8 = (np.arange(P)[:, None] // HEAD_NUM
             == np.arange(8)[None, :]).astype(BF16)
    ident = np.eye(P).astype(BF16)

    in_maps = []
    for c in range(n_cores):
        in_maps.append({
            "x": np.ascontiguousarray(xf[c * rows:(c + 1) * rows]),
            "w_pre": wpre16,
            "b_pre_rep": bpre_rep,
            "w_proj": wproj16,
            "b_proj16_rep": bproj_rep,
            "mask01": mask01,
            "mask8": mask8,
            "ident": ident,
        })
    return in_maps


def kernel(x, W_pre, b_pre, W_proj, b_proj):
    global _PROG
    from concourse.bass_utils import run_bass_kernel_spmd

    if _PROG is None:
        _PROG = _build_program()

    in_maps = _host_inputs(x, W_pre, b_pre, W_proj, b_proj)
    res = run_bass_kernel_spmd(_PROG, in_maps, list(range(N_CORES)))
    out = np.concatenate([res.results[c]["out"] for c in range(N_CORES)],
                         axis=0)
    return out.reshape(*np.asarray(x).shape[:-1], OUTPUT_DIM).astype(np.float32)


# revision 16
# speedup vs baseline: 7.0566x; 2.2342x over previous
"""Trainium2 Bass kernel for the nn_Attention problem.

Math (per flattened batch row b of x):
    qkv = x @ W_pre + b_pre                  # [B, 3*16*128]
    q,k,v -> [B, 16, 128]
    S = softmax(q k^T / sqrt(128), axis=g)   # [B, 16, 16]
    out = (sum_h S_h) . v @ W_proj + 16*b_proj
        = (sigma^T V) @ W_proj + 16*b_proj   with sigma[g] = sum_h S[h, g]

Implementation notes:
  - Data-parallel over 8 NeuronCores: 4096 rows/core (32 tiles of 128 rows).
  - bf16 matmuls with fp32 PSUM accumulation; softmax in fp32.  (fp8
    variants were measured on this hardware: plain fp8 runs ~1.5x faster
    per matmul and DoubleRow fuses 2 k-tiles per instruction at bf16
    instruction cost, but single-fp8 operands cost ~2.6% rms input error
    which lands at ~0.026 rel err end-to-end (budget 2e-2), and any
    error-compensated multi-term split needs >= 3 products per k-chunk,
    which is slower than the 1-product bf16 path.  bf16 is the optimum
    here.)
  - Attention processed in groups of 8 rows so the 128x128 PE array is full:
    stationary/moving operands are contiguous [d=128, (8 rows x 16 heads)]
    slices of PE-transposed, b-major Q/K. Cross-row score blocks are zeroed
    by a multiplicative block-diagonal mask fused with the softmax-denominator
    row sum on the DVE.  Scores for 4 groups share one PSUM bank so a single
    ACT exp() instruction covers 512 columns; reciprocals and bf16 casts are
    batched 4-wide as well, quartering the small-instruction overhead of the
    softmax chain vs one-group-at-a-time processing.
  - sigma = E^T r computed on the PE (contract over the (row,head) partition
    dim), scattered to a block-diagonal [128, 8] operand, and contracted with
    V8 (v rows expanded onto partitions via an SBUF->SBUF reshape DMA) to
    accumulate C^T directly; C^T is then the stationary operand of the final
    projection matmul.
"""

import sys

import numpy as np

for _p in ("/opt/trn_rl_repo",):
    if _p not in sys.path:
        sys.path.insert(0, _p)

import ml_dtypes  # noqa: E402

BF16 = ml_dtypes.bfloat16

HEAD_NUM = 16
HEAD_DIM = 128
INPUT_DIM = 1024
OUTPUT_DIM = 1024
QKV_DIM = 3 * HEAD_NUM * HEAD_DIM  # 6144
N_CORES = 8
B_TOTAL = 64 * 512
ROWS_PER_CORE = B_TOTAL // N_CORES  # 4096
P = 128

_PROG = None


def _build_program(n_tiles=ROWS_PER_CORE // P, repeats=1, bias_free=False):
    from contextlib import ExitStack

    import concourse.tile as tile
    from concourse import bacc, mybir

    dt = mybir.dt
    Alu = mybir.AluOpType
    Act = mybir.ActivationFunctionType

    rows = n_tiles * P
    nc = bacc.Bacc("TRN2", target_bir_lowering=False, debug=False,
                   num_devices=N_CORES)

    x_d = nc.dram_tensor("x", [rows, INPUT_DIM], dt.bfloat16,
                         kind="ExternalInput")
    wpre_d = nc.dram_tensor("w_pre", [INPUT_DIM, QKV_DIM], dt.bfloat16,
                            kind="ExternalInput")
    bpre_d = None if bias_free else nc.dram_tensor(
        "b_pre_rep", [P, QKV_DIM], dt.bfloat16, kind="ExternalInput")
    wproj_d = nc.dram_tensor("w_proj", [HEAD_DIM, OUTPUT_DIM], dt.bfloat16,
                             kind="ExternalInput")
    bproj_d = None if bias_free else nc.dram_tensor(
        "b_proj16_rep", [P, OUTPUT_DIM], dt.float32, kind="ExternalInput")
    mask01_d = nc.dram_tensor("mask01", [P, P], dt.bfloat16,
                              kind="ExternalInput")
    mask8_d = nc.dram_tensor("mask8", [P, 8], dt.bfloat16,
                             kind="ExternalInput")
    ident_d = nc.dram_tensor("ident", [P, P], dt.bfloat16,
                             kind="ExternalInput")
    out_d = nc.dram_tensor("out", [rows, OUTPUT_DIM], dt.float32,
                           kind="ExternalOutput")

    KC = INPUT_DIM // P          # 8 contraction chunks
    NCHUNK = 512                 # psum free width
    SWEEP = 3                    # psum banks used by the qkv matmul
    NSWEEPS = QKV_DIM // (SWEEP * NCHUNK)  # 4
    GROUPS = P // 8              # 16 groups of 8 rows per tile
    GB = 4                       # score groups per psum bank / exp batch
    INV_SQRT_D = 1.0 / float(np.sqrt(HEAD_DIM))

    with tile.TileContext(nc) as tc, ExitStack() as ctx:
        consts = ctx.enter_context(tc.tile_pool(name="consts", bufs=1))
        xt_pool = ctx.enter_context(tc.tile_pool(name="xt", bufs=2))
        qkv_pool = ctx.enter_context(tc.tile_pool(name="qkv", bufs=2))
        qt_pool = ctx.enter_context(tc.tile_pool(name="qt", bufs=2))
        kt_pool = ctx.enter_context(tc.tile_pool(name="kt", bufs=2))
        v8_pool = ctx.enter_context(tc.tile_pool(name="v8", bufs=2))
        att_pool = ctx.enter_context(tc.tile_pool(name="att", bufs=4))
        ct_pool = ctx.enter_context(tc.tile_pool(name="ct", bufs=2))
        out_pool = ctx.enter_context(tc.tile_pool(name="outp", bufs=2))

        qkv_ps = ctx.enter_context(
            tc.tile_pool(name="qkv_ps", bufs=SWEEP, space="PSUM"))
        z_ps = ctx.enter_context(tc.tile_pool(name="z_ps", bufs=2, space="PSUM"))
        tp_ps = ctx.enter_context(tc.tile_pool(name="tp_ps", bufs=2, space="PSUM"))
        ct_ps = ctx.enter_context(tc.tile_pool(name="ct_ps", bufs=1, space="PSUM"))

        # tile 0's x^T load goes first so the first qkv matmul only waits
        # on it plus the first W chunk, not the whole 12.6MB weight load.
        xt0 = xt_pool.tile([P, KC, P], dt.bfloat16, name="xt")
        nc.sync.dma_start_transpose(xt0[:], x_d[0:P, :])
        preloaded_xt = {0: xt0}

        # ---- resident constants ----
        wpre_sb = consts.tile([P, KC, QKV_DIM], dt.bfloat16)
        for k in range(KC):
            eng = nc.sync if k % 2 == 0 else nc.scalar
            eng.dma_start(wpre_sb[:, k, :], wpre_d[k * P:(k + 1) * P, :])
        wproj_sb = consts.tile([P, OUTPUT_DIM], dt.bfloat16)
        nc.sync.dma_start(wproj_sb[:], wproj_d[:, :])
        if not bias_free:
            bpre_sb = consts.tile([P, QKV_DIM], dt.bfloat16)
            nc.sync.dma_start(bpre_sb[:], bpre_d[:, :])
            bproj_sb = consts.tile([P, OUTPUT_DIM], dt.float32)
            nc.sync.dma_start(bproj_sb[:], bproj_d[:, :])
        mask01_sb = consts.tile([P, P], dt.bfloat16)
        nc.sync.dma_start(mask01_sb[:], mask01_d[:, :])
        mask8_sb = consts.tile([P, 8], dt.bfloat16)
        nc.sync.dma_start(mask8_sb[:], mask8_d[:, :])
        ident_sb = consts.tile([P, P], dt.bfloat16)
        nc.sync.dma_start(ident_sb[:], ident_d[:, :])

        state = {}

        def front_gen(t):
            """Emits tile t's qkv matmuls, yielding after each one so the
            caller can interleave the previous tile's attention steps into
            the PE queue; finishes with transposes + the V8 reshape."""
            r0 = t * P
            # x^T tiles via XBAR DMA transpose: xt[d, kc, b] = x[r0+b, kc*128+d]
            if t in preloaded_xt:
                xt = preloaded_xt.pop(t)
            else:
                xt = xt_pool.tile([P, KC, P], dt.bfloat16, name="xt")
                nc.sync.dma_start_transpose(xt[:], x_d[r0:r0 + P, :])

            qkv_sb = qkv_pool.tile([P, QKV_DIM], dt.bfloat16, name="qkv_sb")
            for s in range(NSWEEPS):
                chunks = []
                for c in range(SWEEP):
                    chunks.append(qkv_ps.tile([P, NCHUNK], dt.float32,
                                              name="qkvps", tag="qkvps"))
                for k in range(KC):
                    for c in range(SWEEP):
                        j0 = (s * SWEEP + c) * NCHUNK
                        nc.tensor.matmul(
                            chunks[c][:],
                            lhsT=xt[:, k, :],
                            rhs=wpre_sb[:, k, j0:j0 + NCHUNK],
                            start=(k == 0),
                            stop=(k == KC - 1),
                        )
                        yield
                for c in range(SWEEP):
                    j0 = (s * SWEEP + c) * NCHUNK
                    if bias_free:
                        # pure fp32 psum -> bf16 SBUF cast, split DVE/ACT
                        if (s + c) % 2 == 0:
                            nc.vector.tensor_copy(
                                qkv_sb[:, j0:j0 + NCHUNK], chunks[c][:])
                        else:
                            nc.scalar.copy(
                                qkv_sb[:, j0:j0 + NCHUNK], chunks[c][:])
                    else:
                        # psum fp32 + b_pre -> bf16 SBUF
                        nc.vector.tensor_tensor(
                            qkv_sb[:, j0:j0 + NCHUNK],
                            chunks[c][:],
                            bpre_sb[:, j0:j0 + NCHUNK],
                            Alu.add,
                        )

            # transposed q/k in b-major layout: qt[d, b*16+h] = q[b, h*128+d],
            # so every 8-row group is a contiguous 128-column slice (matmul
            # operands must have a single free dimension). 8 head-transposes
            # share one PSUM bank, drained by a single strided copy.
            qt = qt_pool.tile([P, P, HEAD_NUM], dt.bfloat16, name="qt")
            kt = kt_pool.tile([P, P, HEAD_NUM], dt.bfloat16, name="kt")
            for qk in range(2):
                src_off = 2048 * qk
                dst = (qt, kt)[qk]
                for hb in range(2):
                    h0 = 8 * hb
                    tpb = tp_ps.tile([P, 8 * P], dt.bfloat16, name="tpb",
                                     tag="tpb")
                    for hl in range(8):
                        nc.tensor.transpose(
                            tpb[:, hl * P:(hl + 1) * P],
                            qkv_sb[:, src_off + (h0 + hl) * P:
                                   src_off + (h0 + hl + 1) * P],
                            ident_sb[:])
                    (nc.scalar.copy if (qk + hb) % 2 == 0
                     else nc.vector.tensor_copy)(
                        dst[:, :, h0:h0 + 8],
                        tpb.rearrange("d (h b) -> d b h", b=P))

            # v8[(b_loc, g), grp, d] = v[8*grp + b_loc, g*128 + d]
            v8 = v8_pool.tile([P, GROUPS, HEAD_DIM], dt.bfloat16, name="v8")
            for g in range(GROUPS):
                nc.sync.dma_start(
                    v8[:, g, :],
                    qkv_sb[8 * g:8 * g + 8, 4096:6144].rearrange(
                        "b (g d) -> b g d", d=HEAD_DIM),
                )
            state[t] = (qt, kt, v8)

        def attention_steps(t):
            """Returns (steps, tail): `steps` are callables interleaved with
            the next tile's qkv matmuls.  Scores are produced in batches of
            GB=4 groups sharing one PSUM bank so exp() covers 512 columns per
            ACT instruction; mm2/mm3 trail the batch."""
            qt, kt, v8 = state.pop(t)
            r0 = t * P
            ct_box = {}
            zbank, ems, rbfs, sds = {}, {}, {}, {}

            def mm1(g):
                if g == 0:
                    ct_box["ct"] = ct_ps.tile([P, P], dt.float32, name="ct")
                b = g // GB
                gi = g % GB
                if gi == 0:
                    zbank[b] = z_ps.tile([P, GB * P], dt.float32, name="z4",
                                         tag="z4")
                b0 = 8 * g
                # scores for 8 rows x all head pairs: [(b,h), (b',g)]
                nc.tensor.matmul(
                    zbank[b][:, gi * P:(gi + 1) * P],
                    lhsT=qt[:, b0:b0 + 8, :].rearrange("d b h -> d (b h)"),
                    rhs=kt[:, b0:b0 + 8, :].rearrange("d b h -> d (b h)"),
                    start=True,
                    stop=True,
                )

            def softmax_batch(b):
                # one exp() over the whole 4-group bank, then per-group
                # masked row-sums, one batched reciprocal and one batched
                # fp32->bf16 cast.
                em4 = att_pool.tile([P, GB * P], dt.bfloat16, tag="em4",
                                    name="em4")
                nc.scalar.activation(em4[:], zbank[b][:], Act.Exp,
                                     scale=INV_SQRT_D)
                den4 = att_pool.tile([P, GB], dt.float32, tag="den4",
                                     name="den4")
                emm4 = att_pool.tile([P, GB * P], dt.bfloat16, tag="emm4",
                                     name="emm4")
                for gi in range(GB):
                    nc.vector.scalar_tensor_tensor(
                        out=emm4[:, gi * P:(gi + 1) * P],
                        in0=em4[:, gi * P:(gi + 1) * P],
                        scalar=1.0,
                        in1=mask01_sb[:],
                        op0=Alu.mult, op1=Alu.mult,
                        accum_out=den4[:, gi:gi + 1])
                    ems[b * GB + gi] = emm4
                r4 = att_pool.tile([P, GB], dt.float32, tag="r4", name="r4")
                nc.vector.reciprocal(r4[:], den4[:])
                rbf4 = att_pool.tile([P, GB], dt.bfloat16, tag="rbf4",
                                     name="rbf4")
                nc.vector.tensor_copy(rbf4[:], r4[:])
                for gi in range(GB):
                    rbfs[b * GB + gi] = rbf4[:, gi:gi + 1]

            def mm2(g):
                b = g // GB
                gi = g % GB
                em = ems.pop(g)
                # sigma[(b,g)] = sum_{(b,h)} em[(b,h),(b,g)] * r[(b,h)]
                sig = zbank[b][:, gi * P:gi * P + 1]  # dead z psum slot
                nc.tensor.matmul(sig, lhsT=em[:, gi * P:(gi + 1) * P],
                                 rhs=rbfs.pop(g), start=True, stop=True)
                sd = att_pool.tile([P, 8], dt.bfloat16, tag="sd", name="sd")
                nc.vector.tensor_scalar(sd[:], mask8_sb[:], sig, None,
                                        Alu.mult)
                sds[g] = sd

            def mm3(g):
                b0 = 8 * g
                # C^T[:, rows of this group] += sigma-weighted V rows
                nc.tensor.matmul(ct_box["ct"][:, b0:b0 + 8],
                                 lhsT=v8[:, g, :],
                                 rhs=sds.pop(g)[:], start=True, stop=True)
                if g % GB == GB - 1:
                    zbank.pop(g // GB, None)

            steps = []
            for g in range(GROUPS + 5):
                def step(g=g):
                    # mm2 lags mm1 by 4 (its exp batch), mm3 by one more.
                    if 4 <= g < GROUPS + 4:
                        mm2(g - 4)
                    if g < GROUPS:
                        mm1(g)
                        if g % GB == GB - 1:
                            softmax_batch(g // GB)
                    if g >= 5:
                        mm3(g - 5)
                steps.append(step)

            def tail():
                ct_sb = ct_pool.tile([P, P], dt.bfloat16, name="ct_sb")
                nc.scalar.copy(ct_sb[:], ct_box["ct"][:])
                out_sb = out_pool.tile([P, OUTPUT_DIM], dt.float32,
                                       name="out_sb")
                for c in range(OUTPUT_DIM // NCHUNK):
                    o_ps = qkv_ps.tile([P, NCHUNK], dt.float32, name="o_ps",
                                       tag="qkvps")
                    nc.tensor.matmul(
                        o_ps[:],
                        lhsT=ct_sb[:],
                        rhs=wproj_sb[:, c * NCHUNK:(c + 1) * NCHUNK],
                        start=True,
                        stop=True,
                    )
                    if bias_free:
                        # plain fp32 psum -> SBUF copy (DMA cannot read PSUM)
                        (nc.vector.tensor_copy if c % 2 == 0
                         else nc.scalar.copy)(
                            out_sb[:, c * NCHUNK:(c + 1) * NCHUNK],
                            o_ps[:])
                    else:
                        nc.vector.tensor_tensor(
                            out_sb[:, c * NCHUNK:(c + 1) * NCHUNK],
                            o_ps[:],
                            bproj_sb[:, c * NCHUNK:(c + 1) * NCHUNK],
                            Alu.add,
                        )
                nc.sync.dma_start(out_d[r0:r0 + P, :], out_sb[:])

            return steps, tail

        # software pipeline: tile t's attention steps are interleaved into
        # tile t+1's qkv matmul stream so the PE stays busy while the
        # softmax chains drain on ACT/DVE. repeats>1 re-runs the whole pass
        # (same outputs) for benchmarking.
        prev = None
        for _r in range(repeats):
            for t in range(n_tiles):
                steps, tail = attention_steps(prev) if prev is not None \
                    else ([], None)
                si = 0
                yi = 0
                for _ in front_gen(t):
                    yi += 1
                    if si < len(steps) and yi % 4 == 0:
                        steps[si]()
                        si += 1
                while si < len(steps):
                    steps[si]()
                    si += 1
                if tail is not None:
                    tail()
                prev = t
        steps, tail = attention_steps(prev)
        for s in steps:
            s()
        tail()

    nc.compile()
    return nc


def _host_inputs(x, W_pre, b_pre, W_proj, b_proj, n_tiles=ROWS_PER_CORE // P,
                 n_cores=N_CORES, bias_free=None):
    if bias_free is None:
        bias_free = not (np.any(np.asarray(b_pre))
                         or np.any(np.asarray(b_proj)))
    rows = n_tiles * P
    xf = np.ascontiguousarray(np.asarray(x, dtype=np.float32)
                              .reshape(-1, INPUT_DIM)).astype(BF16)
    wpre16 = np.asarray(W_pre, dtype=np.float32).astype(BF16)
    wproj16 = np.asarray(W_proj, dtype=np.float32).astype(BF16)
    bpre_rep = np.broadcast_to(
        np.asarray(b_pre, dtype=np.float32).astype(BF16)[None, :],
        (P, QKV_DIM)).copy()
    bproj_rep = np.broadcast_to(
        (16.0 * np.asarray(b_proj, dtype=np.float32))[None, :],
        (P, OUTPUT_DIM)).copy()
    pi = np.arange(P)[:, None] // HEAD_NUM
    fi = np.arange(P)[None, :] // HEAD_NUM
    mask01 = (pi == fi).astype(BF16)
    mask8 = (np.arange(P)[:, None] // HEAD_NUM
             == np.arange(8)[None, :]).astype(BF16)
    ident = np.eye(P).astype(BF16)

    in_maps = []
    for c in range(n_cores):
        m = {
            "x": np.ascontiguousarray(xf[c * rows:(c + 1) * rows]),
            "w_pre": wpre16,
            "w_proj": wproj16,
            "mask01": mask01,
            "mask8": mask8,
            "ident": ident,
        }
        if not bias_free:
            m["b_pre_rep"] = bpre_rep
            m["b_proj16_rep"] = bproj_rep
        in_maps.append(m)
    return in_maps


def kernel(x, W_pre, b_pre, W_proj, b_proj):
    global _PROG
    from concourse.bass_utils import run_bass_kernel_spmd

    # the biases are folded into the PSUM drains; when they are all-zero
    # (as in this problem's inputs) a leaner program skips those adds and
    # writes the projection PSUM straight to DRAM.
    bias_free = not (np.any(np.asarray(b_pre)) or np.any(np.asarray(b_proj)))
    if _PROG is None or _PROG[1] != bias_free:
        _PROG = (_build_program(bias_free=bias_free), bias_free)

    in_maps = _host_inputs(x, W_pre, b_pre, W_proj, b_proj,
                           bias_free=bias_free)
    res = run_bass_kernel_spmd(_PROG[0], in_maps, list(range(N_CORES)))
    out = np.concatenate([res.results[c]["out"] for c in range(N_CORES)],
                         axis=0)
    return out.reshape(*np.asarray(x).shape[:-1], OUTPUT_DIM).astype(np.float32)
